# revision 8
# baseline (speedup 1.0000x reference)
"""Trainium2 Bass kernel for nn_AttCM: 1x1-conv stem -> (two 3x3 convs) +
(single-head spatial attention), alpha/beta combined.

Sharding: 8 cores = 4 samples x 2 halves of the attention key axis (n).
Each core computes the full stem + q for its sample (cheap), its n-half of
S = k^T q with full softmax rows (softmax axis is m, fully local), a partial
attn_out = (v/l) @ exp(S) (host adds the two partials), and half of the 3x3
conv branch rows. No cross-core communication; the host applies
alpha*conv + beta*attn and the inverse of the per-core pixel roll.

SPMD trick: all 8 cores run one graph. Per-core behavior comes from data:
  - xq is the sample pixel-rolled by -2048*h so the core's k/v half is always
    columns [0, 2048) of its local x3; the attention output columns are rolled
    back on the host.
  - xc is a 36-row window of the sample (host zero-padded at image borders)
    so the conv branch always computes local output rows 2..33.
  - mtop/mbot (0.0 or 1.0 per core) zero the stem-of-zero padding rows that
    a true conv 'SAME' zero-pad requires.

All matmul inputs are bf16 (fp32 PSUM accumulation); measured rel_l2 vs the
fp32 reference ~2e-3.
"""

import numpy as np
import ml_dtypes

_CACHE = {}

B, C, H, W = 4, 256, 64, 64
N = H * W            # 4096 pixels
NH = N // 2          # per-core attention key half
NB = 16              # n-blocks of 128 rows per core


def _build_nc():
    from contextlib import ExitStack

    import concourse.mybir as mybir
    import concourse.tile as tile
    from concourse import bacc

    f32 = mybir.dt.float32
    bf16 = mybir.dt.bfloat16
    AF = mybir.ActivationFunctionType
    AX = mybir.AxisListType

    nc = bacc.Bacc("TRN2", target_bir_lowering=False, debug=False)

    def din(name, shape, dt=bf16):
        return nc.dram_tensor(name, shape, dt, kind="ExternalInput").ap()

    xq_d = din("xq", [3, N])
    xc_d = din("xc", [3, 36 * 64])
    w1t_d = din("w1t", [3, 64])
    b1_d = din("b1", [64, 1], f32)
    w2t_d = din("w2t", [64, 128])
    b2_d = din("b2", [128, 1], f32)
    w3t_d = din("w3t", [128, 256])
    b3_d = din("b3", [128, 2], f32)
    wqt_d = din("wqt", [128, 2, 256])
    bq_d = din("bq", [128, 2], f32)
    wkt_d = din("wkt", [128, 2, 256])
    bk_d = din("bk", [128, 2], f32)
    wvt_d = din("wvt", [128, 2, 256])
    bv_d = din("bv", [1, 256])
    wb1_d = din("wb1", [128, 2, 9, 256])
    bb1_d = din("bb1", [128, 2], f32)
    wb2_d = din("wb2", [128, 2, 9, 256])
    bb2_d = din("bb2", [128, 2], f32)
    mtop_d = din("mtop", [128, 1], f32)
    mbot_d = din("mbot", [128, 1], f32)

    oa_d = nc.dram_tensor("out_attn", [C, N], f32, kind="ExternalOutput").ap()
    oc_d = nc.dram_tensor("out_conv", [C, 32 * 64], f32, kind="ExternalOutput").ap()

    with tile.TileContext(nc) as tc, ExitStack() as ctx:
        singles = ctx.enter_context(tc.tile_pool(name="singles", bufs=1))
        ps = ctx.enter_context(tc.tile_pool(name="ps", bufs=8, space="PSUM"))
        big = ctx.enter_context(tc.tile_pool(name="big", bufs=1))

        def load(d, shape, dt=bf16, tag=None):
            nm = d.tensor.name + "_sb"
            t = (singles.tile(shape, dt, tag=tag, name=nm) if tag
                 else singles.tile(shape, dt, name=nm))
            nc.sync.dma_start(out=t, in_=d)
            return t

        w1t = load(w1t_d, [3, 64])
        b1 = load(b1_d, [64, 1], f32)
        w2t = load(w2t_d, [64, 128])
        b2 = load(b2_d, [128, 1], f32)
        w3t = load(w3t_d, [128, 256])
        b3 = load(b3_d, [128, 2], f32)
        wqt = load(wqt_d, [128, 2, 256])
        bq = load(bq_d, [128, 2], f32)
        wkt = load(wkt_d, [128, 2, 256])
        bk = load(bk_d, [128, 2], f32)
        wvt = load(wvt_d, [128, 2, 256])
        bv = load(bv_d, [1, 256])
        bb1 = load(bb1_d, [128, 2], f32)
        bb2 = load(bb2_d, [128, 2], f32)
        mtop = load(mtop_d, [128, 1], f32)
        mbot = load(mbot_d, [128, 1], f32)
        ones = singles.tile([1, 128], bf16)
        nc.vector.memset(ones, 1.0)
        lall = singles.tile([128, NB], f32)
        rl = singles.tile([128, NB], f32)

        # ---- stem on the rolled full sample (feeds q, k, v) ----
        xq = big.tile([3, N], bf16, tag="x_in")
        nc.sync.dma_start(out=xq, in_=xq_d)
        h1 = big.tile([64, N], bf16, tag="h1")
        for t in range(8):
            s = slice(t * 512, (t + 1) * 512)
            p = ps.tile([64, 512], f32, tag="ps")
            nc.tensor.matmul(p, w1t, xq[:, s], start=True, stop=True)
            nc.scalar.activation(h1[:, s], p, AF.Relu, bias=b1)
        h2 = big.tile([128, N], bf16, tag="h2")
        for t in range(8):
            s = slice(t * 512, (t + 1) * 512)
            p = ps.tile([128, 512], f32, tag="ps")
            nc.tensor.matmul(p, w2t, h1[:, s], start=True, stop=True)
            nc.scalar.activation(h2[:, s], p, AF.Relu, bias=b2)
        x3q = big.tile([128, 2, N], bf16, tag="x3q")
        for cc in range(2):
            for t in range(8):
                s = slice(t * 512, (t + 1) * 512)
                p = ps.tile([128, 512], f32, tag="ps")
                nc.tensor.matmul(
                    p, w3t[:, cc * 128 : (cc + 1) * 128], h2[:, s],
                    start=True, stop=True,
                )
                nc.scalar.activation(
                    x3q[:, cc, s], p, AF.Relu, bias=b3[:, cc : cc + 1]
                )

        # ---- q (full m), k (local n half), vT (local n half, transposed) ----
        q = big.tile([128, 2, N], bf16, tag="q")
        for cc in range(2):
            for t in range(8):
                s = slice(t * 512, (t + 1) * 512)
                p = ps.tile([128, 512], f32, tag="ps")
                for ki in range(2):
                    nc.tensor.matmul(
                        p, wqt[:, ki, cc * 128 : (cc + 1) * 128], x3q[:, ki, s],
                        start=(ki == 0), stop=(ki == 1),
                    )
                nc.scalar.activation(
                    q[:, cc, s], p, AF.Identity, bias=bq[:, cc : cc + 1]
                )
        k_ = big.tile([128, 2, NH], bf16, tag="k")
        for cc in range(2):
            for t in range(4):
                s = slice(t * 512, (t + 1) * 512)
                p = ps.tile([128, 512], f32, tag="ps")
                for ki in range(2):
                    nc.tensor.matmul(
                        p, wkt[:, ki, cc * 128 : (cc + 1) * 128], x3q[:, ki, s],
                        start=(ki == 0), stop=(ki == 1),
                    )
                nc.scalar.activation(
                    k_[:, cc, s], p, AF.Identity, bias=bk[:, cc : cc + 1]
                )
        # vT[n, c] = sum_ci x3[ci, n] WvT[ci, c] + bv[c]  (bias via K=1 matmul)
        vT = big.tile([128, NB, 256], bf16, tag="vT")
        for nb in range(NB):
            nsl = slice(nb * 128, (nb + 1) * 128)
            p = ps.tile([128, 256], f32, tag="ps")
            nc.tensor.matmul(p, x3q[:, 0, nsl], wvt[:, 0, :], start=True, stop=False)
            nc.tensor.matmul(p, x3q[:, 1, nsl], wvt[:, 1, :], start=False, stop=False)
            nc.tensor.matmul(p, ones, bv, start=False, stop=True)
            nc.vector.tensor_copy(vT[:, nb, :], p)

        # ---- stem on the conv window (local rows 0..35 incl. border pads) ----
        xc = big.tile([3, 36 * 64], bf16, tag="x_in")
        nc.sync.dma_start(out=xc, in_=xc_d)
        CT = [(0, 8), (8, 8), (16, 8), (24, 8), (32, 4)]
        h1c = big.tile([64, 36 * 64], bf16, tag="h1")
        for r0, nr in CT:
            s = slice(r0 * 64, (r0 + nr) * 64)
            p = ps.tile([64, 512], f32, tag="ps")
            nc.tensor.matmul(p[:, : nr * 64], w1t, xc[:, s], start=True, stop=True)
            nc.scalar.activation(h1c[:, s], p[:, : nr * 64], AF.Relu, bias=b1)
        h2c = big.tile([128, 36 * 64], bf16, tag="h2")
        for r0, nr in CT:
            s = slice(r0 * 64, (r0 + nr) * 64)
            p = ps.tile([128, 512], f32, tag="ps")
            nc.tensor.matmul(p[:, : nr * 64], w2t, h1c[:, s], start=True, stop=True)
            nc.scalar.activation(h2c[:, s], p[:, : nr * 64], AF.Relu, bias=b2)
        # x3c in 66-wide zero-padded layout for the 3x3 taps
        x3c = big.tile([128, 2, 36, 66], bf16, tag="x3c")
        nc.vector.memset(x3c, 0.0)
        for cc in range(2):
            for r0, nr in CT:
                s = slice(r0 * 64, (r0 + nr) * 64)
                p = ps.tile([128, 512], f32, tag="ps")
                nc.tensor.matmul(
                    p[:, : nr * 64], w3t[:, cc * 128 : (cc + 1) * 128], h2c[:, s],
                    start=True, stop=True,
                )
                nc.scalar.activation(
                    x3c[:, cc, r0 : r0 + nr, 1:65], p[:, : nr * 64],
                    AF.Relu, bias=b3[:, cc : cc + 1],
                )
        # zero the stem-of-zero border rows (true 'SAME' pad is zero in x3)
        for cc in range(2):
            nc.vector.tensor_scalar_mul(x3c[:, cc, 0:2, :], x3c[:, cc, 0:2, :], mtop)
            nc.vector.tensor_scalar_mul(x3c[:, cc, 34:36, :], x3c[:, cc, 34:36, :], mbot)

        # ---- conv1 (3x3, relu): y1 local rows 1..34 stored at y1p rows 0..33 ----
        wb1 = load(wb1_d, [128, 2, 9, 256], tag="wb")
        y1p0 = big.tile([128, 34, 66], bf16, tag="h1")
        y1p1 = big.tile([128, 34, 66], bf16, tag="x_in")
        y1p_ = lambda ki: y1p0 if ki == 0 else y1p1
        nc.vector.memset(y1p0, 0.0)
        nc.vector.memset(y1p1, 0.0)
        C1T = [(1, 8), (9, 8), (17, 8), (25, 8), (33, 2)]
        for cc in range(2):
            for r0, nr in C1T:
                p = ps.tile([128, 512], f32, tag="ps")
                first = True
                for ki in range(2):
                    for tap in range(9):
                        dh, dw = tap // 3, tap % 3
                        rhs = x3c[:, ki, r0 - 1 + dh : r0 - 1 + dh + nr, dw : dw + 64]
                        nc.tensor.matmul(
                            p[:, : nr * 64],
                            wb1[:, ki, tap, cc * 128 : (cc + 1) * 128],
                            rhs,
                            start=first, stop=(ki == 1 and tap == 8),
                        )
                        first = False
                nc.scalar.activation(
                    y1p_(cc)[:, r0 - 1 : r0 - 1 + nr, 1:65], p[:, : nr * 64],
                    AF.Relu, bias=bb1[:, cc : cc + 1],
                )
        for cc in range(2):
            nc.vector.tensor_scalar_mul(y1p_(cc)[:, 0, :], y1p_(cc)[:, 0, :], mtop)
            nc.vector.tensor_scalar_mul(y1p_(cc)[:, 33, :], y1p_(cc)[:, 33, :], mbot)

        # ---- conv2 (3x3, bias, no relu): out local rows 2..33 ----
        wb2 = load(wb2_d, [128, 2, 9, 256], tag="wb")
        C2T = [(2, 8), (10, 8), (18, 8), (26, 8)]
        for cc in range(2):
            for r0, nr in C2T:
                p = ps.tile([128, 512], f32, tag="ps")
                first = True
                for ki in range(2):
                    for tap in range(9):
                        dh, dw = tap // 3, tap % 3
                        rhs = y1p_(ki)[:, r0 - 2 + dh : r0 - 2 + dh + nr, dw : dw + 64]
                        nc.tensor.matmul(
                            p,
                            wb2[:, ki, tap, cc * 128 : (cc + 1) * 128],
                            rhs,
                            start=first, stop=(ki == 1 and tap == 8),
                        )
                        first = False
                st = big.tile([128, 512], f32, tag=("h2" if (r0 // 8 + cc) % 2 else "x3c"), name="st_c")
                nc.vector.tensor_scalar_add(st, p, bb2[:, cc : cc + 1])
                nc.sync.dma_start(
                    out=oc_d[cc * 128 : (cc + 1) * 128, (r0 - 2) * 64 : (r0 - 2) * 64 + 512],
                    in_=st,
                )

        # ---- S = k^T q per 128-row n-block; P = exp(S) (no max: |S| < 0.5);
        #      l[n] = sum_m P via activation accumulate ----
        P0 = big.tile([128, NB // 2, N], bf16, tag="x3q")
        P1 = big.tile([128, NB // 2, N], bf16, tag="P1")

        def P_(nb):
            return (P0 if nb < NB // 2 else P1)[:, nb % (NB // 2), :]

        for nb in range(NB):
            nsl = slice(nb * 128, (nb + 1) * 128)
            lp = singles.tile([128, 8], f32, tag="lp", bufs=4, name="lp")
            for mt in range(8):
                s = slice(mt * 512, (mt + 1) * 512)
                p = ps.tile([128, 512], f32, tag="ps")
                for ki in range(2):
                    nc.tensor.matmul(
                        p, k_[:, ki, nsl], q[:, ki, s],
                        start=(ki == 0), stop=(ki == 1),
                    )
                nc.scalar.activation(
                    P_(nb)[:, s], p, AF.Exp, accum_out=lp[:, mt : mt + 1]
                )
            nc.vector.reduce_sum(out=lall[:, nb : nb + 1], in_=lp, axis=AX.X)

        # ---- fold 1/l into vT (softmax normalizer on the contracted axis) ----
        nc.vector.reciprocal(rl, lall)
        for nb in range(NB):
            nc.vector.tensor_scalar_mul(vT[:, nb, :], vT[:, nb, :], rl[:, nb : nb + 1])

        # ---- attn_out partial = (v/l) @ P ----
        for cc in range(2):
            for mt in range(8):
                s = slice(mt * 512, (mt + 1) * 512)
                p = ps.tile([128, 512], f32, tag="ps")
                for nb in range(NB):
                    nc.tensor.matmul(
                        p, vT[:, nb, cc * 128 : (cc + 1) * 128], P_(nb)[:, s],
                        start=(nb == 0), stop=(nb == NB - 1),
                    )
                st = big.tile([128, 512], f32, tag=("h2" if mt % 2 else "x3c"), name="st_a")
                if mt % 2 == 0:
                    nc.vector.tensor_copy(st, p)
                else:
                    nc.scalar.copy(st, p)
                nc.sync.dma_start(
                    out=oa_d[cc * 128 : (cc + 1) * 128, s], in_=st
                )

    nc.compile()
    return nc


def _get_nc():
    if "nc" not in _CACHE:
        _CACHE["nc"] = _build_nc()
    return _CACHE["nc"]


def _make_in_maps(x, w1, b1, w2, b2, w3, b3, wb1, bb1, wb2, bb2,
                  wq, bq, wk, bk, wv, bv):
    bfc = lambda a: np.ascontiguousarray(np.asarray(a, np.float32).astype(ml_dtypes.bfloat16))
    f32c = lambda a: np.ascontiguousarray(np.asarray(a, np.float32))

    def qkv_t(w):  # [O, CI] -> lhsT/rhs chunks [128, 2, 256]
        return bfc(np.asarray(w, np.float32).T.reshape(2, 128, 256).transpose(1, 0, 2))

    def conv_t(wb):  # [O, I, 3, 3] -> [128 kip, 2 ki, 9 tap, 256 o]
        a = np.asarray(wb, np.float32).transpose(1, 0, 2, 3)  # [I, O, 3, 3]
        a = a.reshape(2, 128, 256, 9)                          # [ki, kip, o, tap]
        return bfc(a.transpose(1, 0, 3, 2))                    # [kip, ki, tap, o]

    def bias2(b):  # [256] -> [128, 2] (col cc = chunk cc)
        return f32c(np.asarray(b, np.float32).reshape(2, 128).T)

    common = {
        "w1t": bfc(np.asarray(w1).T), "b1": f32c(np.asarray(b1).reshape(64, 1)),
        "w2t": bfc(np.asarray(w2).T), "b2": f32c(np.asarray(b2).reshape(128, 1)),
        "w3t": bfc(np.asarray(w3).T), "b3": bias2(b3),
        "wqt": qkv_t(wq), "bq": bias2(bq),
        "wkt": qkv_t(wk), "bk": bias2(bk),
        "wvt": qkv_t(wv), "bv": bfc(np.asarray(bv).reshape(1, 256)),
        "wb1": conv_t(wb1), "bb1": bias2(bb1),
        "wb2": conv_t(wb2), "bb2": bias2(bb2),
    }

    xf = np.asarray(x, np.float32).reshape(B, 3, N)
    in_maps = []
    for core in range(8):
        b, h = core // 2, core % 2
        xq = bfc(np.roll(xf[b], -NH * h, axis=1))
        # conv window: global rows [32h-2, 32h+34), zero outside the image
        xi = xf[b].reshape(3, H, W)
        xc = np.zeros((3, 36, W), np.float32)
        g0 = 32 * h - 2
        lo, hi = max(0, g0), min(H, g0 + 36)
        xc[:, lo - g0 : hi - g0, :] = xi[:, lo:hi, :]
        m = np.full((128, 1), 1.0, np.float32)
        z = np.full((128, 1), 0.0, np.float32)
        in_maps.append(dict(
            common,
            xq=xq,
            xc=bfc(xc.reshape(3, 36 * 64)),
            mtop=(z if h == 0 else m),
            mbot=(m if h == 0 else z),
        ))
    return in_maps


def _gather(results, alpha, beta):
    a, bt = float(alpha), float(beta)
    out = np.empty((B, C, H, W), np.float32)
    for b in range(B):
        r0, r1 = results[2 * b], results[2 * b + 1]
        attn = r0["out_attn"] + np.roll(r1["out_attn"], NH, axis=1)
        conv = np.concatenate(
            [r0["out_conv"].reshape(C, 32, W), r1["out_conv"].reshape(C, 32, W)],
            axis=1,
        )
        out[b] = a * conv + bt * attn.reshape(C, H, W)
    return out


def _run(inputs, trace=False, **kw):
    from concourse import bass_utils

    nc = _get_nc()
    in_maps = _make_in_maps(
        inputs["x"], inputs["w1"], inputs["b1"], inputs["w2"], inputs["b2"],
        inputs["w3"], inputs["b3"], inputs["wb1"], inputs["bb1"],
        inputs["wb2"], inputs["bb2"], inputs["wq"], inputs["bq"],
        inputs["wk"], inputs["bk"], inputs["wv"], inputs["bv"],
    )
    res = bass_utils.run_bass_kernel_spmd(
        nc, in_maps, core_ids=list(range(8)), trace=trace, **kw
    )
    return _gather(res.results, inputs["alpha"], inputs["beta"]), res


def kernel(**inputs):
    out, _ = _run(inputs, trace=False)
    return out


# revision 10
# speedup vs baseline: 1.1223x; 1.1223x over previous
"""Trainium2 Bass kernel for nn_AttCM: 1x1-conv stem -> (two 3x3 convs) +
(single-head spatial attention), alpha/beta combined.

Sharding: 8 cores = 4 samples x 2 halves of the attention key axis (n).
Each core computes the full stem + q for its sample (cheap), its n-half of
S = k^T q with full softmax rows (softmax axis is m, fully local), a partial
attn_out = (v/l) @ exp(S) (host adds the two partials), and half of the 3x3
conv branch rows. No cross-core communication; the host applies
alpha*conv + beta*attn and the inverse of the per-core pixel roll.

SPMD trick: all 8 cores run one graph. Per-core behavior comes from data:
  - xq is the sample pixel-rolled by -2048*h so the core's k/v half is always
    columns [0, 2048) of its local x3; the attention output columns are rolled
    back on the host.
  - xc is a 36-row window of the sample (host zero-padded at image borders)
    so the conv branch always computes local output rows 2..33.
  - mtop/mbot (0.0 or 1.0 per core) zero the stem-of-zero padding rows that
    a true conv 'SAME' zero-pad requires.

All matmul inputs are bf16 (fp32 PSUM accumulation); measured rel_l2 vs the
fp32 reference ~2e-3.

Schedule notes: the S loop is ScalarE-bound (exp of 8.4M elements/core), so
the 3x3 conv matmul groups are interleaved between S blocks to keep TensorE
busy while ScalarE drains exp; PSUM is managed as 4 slots of 2 banks each
(2 for S ping-pong, 2 for the interleaved conv groups).
"""

import numpy as np
import ml_dtypes

_CACHE = {}

B, C, H, W = 4, 256, 64, 64
N = H * W            # 4096 pixels
NH = N // 2          # per-core attention key half
NB = 16              # n-blocks of 128 rows per core


def _build_nc():
    from contextlib import ExitStack

    import concourse.mybir as mybir
    import concourse.tile as tile
    from concourse import bacc

    f32 = mybir.dt.float32
    bf16 = mybir.dt.bfloat16
    AF = mybir.ActivationFunctionType
    AX = mybir.AxisListType

    nc = bacc.Bacc("TRN2", target_bir_lowering=False, debug=False)

    def din(name, shape, dt=bf16):
        return nc.dram_tensor(name, shape, dt, kind="ExternalInput").ap()

    xq_d = din("xq", [3, N])
    xc_d = din("xc", [3, 36 * 64])
    w1t_d = din("w1t", [3, 64])
    b1_d = din("b1", [64, 1], f32)
    w2t_d = din("w2t", [64, 128])
    b2_d = din("b2", [128, 1], f32)
    w3t_d = din("w3t", [128, 256])
    b3_d = din("b3", [128, 2], f32)
    wqt_d = din("wqt", [128, 2, 256])
    bq_d = din("bq", [128, 2], f32)
    wkt_d = din("wkt", [128, 2, 256])
    bk_d = din("bk", [128, 2], f32)
    wvt_d = din("wvt", [128, 2, 256])
    bv_d = din("bv", [1, 256])
    wb1_d = din("wb1", [128, 2, 9, 256])
    bb1_d = din("bb1", [128, 2], f32)
    wb2_d = din("wb2", [128, 2, 9, 256])
    bb2_d = din("bb2", [128, 2], f32)
    mtop_d = din("mtop", [128, 1], f32)
    mbot_d = din("mbot", [128, 1], f32)

    oa_d = nc.dram_tensor("out_attn", [C, N], f32, kind="ExternalOutput").ap()
    oc_d = nc.dram_tensor("out_conv", [C, 32 * 64], f32, kind="ExternalOutput").ap()

    with tile.TileContext(nc) as tc, ExitStack() as ctx:
        singles = ctx.enter_context(tc.tile_pool(name="singles", bufs=1))
        ps = ctx.enter_context(tc.tile_pool(name="ps", bufs=4, space="PSUM"))
        big = ctx.enter_context(tc.tile_pool(name="big", bufs=1))

        def load(d, shape, dt=bf16, tag=None):
            nm = d.tensor.name + "_sb"
            t = (singles.tile(shape, dt, tag=tag, name=nm) if tag
                 else singles.tile(shape, dt, name=nm))
            nc.sync.dma_start(out=t, in_=d)
            return t

        w1t = load(w1t_d, [3, 64])
        b1 = load(b1_d, [64, 1], f32)
        w2t = load(w2t_d, [64, 128])
        b2 = load(b2_d, [128, 1], f32)
        w3t = load(w3t_d, [128, 256])
        b3 = load(b3_d, [128, 2], f32)
        wqt = load(wqt_d, [128, 2, 256])
        bq = load(bq_d, [128, 2], f32)
        wkt = load(wkt_d, [128, 2, 256])
        bk = load(bk_d, [128, 2], f32)
        wvt = load(wvt_d, [128, 2, 256])
        bv = load(bv_d, [1, 256])
        bb1 = load(bb1_d, [128, 2], f32)
        bb2 = load(bb2_d, [128, 2], f32)
        mtop = load(mtop_d, [128, 1], f32)
        mbot = load(mbot_d, [128, 1], f32)
        ones = singles.tile([1, 128], bf16)
        nc.vector.memset(ones, 1.0)
        lall = singles.tile([128, NB], f32)
        rl = singles.tile([128, NB], f32)

        # ---- stem on the rolled full sample (feeds q, k, v) ----
        xq = big.tile([3, N], bf16, tag="x_in")
        nc.sync.dma_start(out=xq, in_=xq_d)
        h1 = big.tile([64, N], bf16, tag="h1")
        for t in range(4):
            p = ps.tile([64, 1024], f32, tag="ps", name="p_h1")
            for su in range(2):
                nc.tensor.matmul(
                    p[:, su * 512 : (su + 1) * 512], w1t,
                    xq[:, t * 1024 + su * 512 : t * 1024 + (su + 1) * 512],
                    start=True, stop=True,
                )
            nc.scalar.activation(h1[:, t * 1024 : (t + 1) * 1024], p, AF.Relu, bias=b1)
        h2 = big.tile([128, N], bf16, tag="h2")
        for t in range(4):
            p = ps.tile([128, 1024], f32, tag="ps", name="p_h2")
            for su in range(2):
                nc.tensor.matmul(
                    p[:, su * 512 : (su + 1) * 512], w2t,
                    h1[:, t * 1024 + su * 512 : t * 1024 + (su + 1) * 512],
                    start=True, stop=True,
                )
            nc.scalar.activation(h2[:, t * 1024 : (t + 1) * 1024], p, AF.Relu, bias=b2)
        x3q = big.tile([128, 2, N], bf16, tag="x3q")
        for cc in range(2):
            for t in range(4):
                p = ps.tile([128, 1024], f32, tag="ps", name="p_x3q")
                for su in range(2):
                    nc.tensor.matmul(
                        p[:, su * 512 : (su + 1) * 512],
                        w3t[:, cc * 128 : (cc + 1) * 128],
                        h2[:, t * 1024 + su * 512 : t * 1024 + (su + 1) * 512],
                        start=True, stop=True,
                    )
                nc.scalar.activation(
                    x3q[:, cc, t * 1024 : (t + 1) * 1024], p,
                    AF.Relu, bias=b3[:, cc : cc + 1],
                )

        # ---- q (full m), k (local n half), vT (local n half, transposed) ----
        q = big.tile([128, 2, N], bf16, tag="q")
        for cc in range(2):
            for t in range(4):
                p = ps.tile([128, 1024], f32, tag="ps", name="p_q")
                for ki in range(2):
                    for su in range(2):
                        nc.tensor.matmul(
                            p[:, su * 512 : (su + 1) * 512],
                            wqt[:, ki, cc * 128 : (cc + 1) * 128],
                            x3q[:, ki, t * 1024 + su * 512 : t * 1024 + (su + 1) * 512],
                            start=(ki == 0), stop=(ki == 1),
                        )
                nc.vector.tensor_scalar_add(
                    q[:, cc, t * 1024 : (t + 1) * 1024], p, bq[:, cc : cc + 1]
                )
        k_ = big.tile([128, 2, NH], bf16, tag="k")
        for cc in range(2):
            for t in range(2):
                p = ps.tile([128, 1024], f32, tag="ps", name="p_k")
                for ki in range(2):
                    for su in range(2):
                        nc.tensor.matmul(
                            p[:, su * 512 : (su + 1) * 512],
                            wkt[:, ki, cc * 128 : (cc + 1) * 128],
                            x3q[:, ki, t * 1024 + su * 512 : t * 1024 + (su + 1) * 512],
                            start=(ki == 0), stop=(ki == 1),
                        )
                nc.vector.tensor_scalar_add(
                    k_[:, cc, t * 1024 : (t + 1) * 1024], p, bk[:, cc : cc + 1]
                )
        # vT[n, c] = sum_ci x3[ci, n] WvT[ci, c] + bv[c]  (bias via K=1 matmul)
        vT = big.tile([128, NB, 256], bf16, tag="vT")
        for g in range(4):
            p = ps.tile([128, 1024], f32, tag="ps", name="p_vT")
            for j in range(4):
                nb = g * 4 + j
                nsl = slice(nb * 128, (nb + 1) * 128)
                o = slice(j * 256, (j + 1) * 256)
                nc.tensor.matmul(p[:, o], x3q[:, 0, nsl], wvt[:, 0, :], start=True, stop=False)
                nc.tensor.matmul(p[:, o], x3q[:, 1, nsl], wvt[:, 1, :], start=False, stop=False)
                nc.tensor.matmul(p[:, o], ones, bv, start=False, stop=True)
            nc.vector.tensor_copy(vT[:, g * 4 : (g + 1) * 4, :], p)

        # ---- stem on the conv window (local rows 0..35 incl. border pads) ----
        xc = big.tile([3, 36 * 64], bf16, tag="x_in")
        nc.sync.dma_start(out=xc, in_=xc_d)
        CT = [(0, 16), (16, 16), (32, 4)]  # (row0, nrows)
        h1c = big.tile([64, 36 * 64], bf16, tag="h1")
        for r0, nr in CT:
            w = nr * 64
            p = ps.tile([64, 1024], f32, tag="ps", name="p_h1c")
            for su in range(0, w, 512):
                e = min(su + 512, w)
                nc.tensor.matmul(
                    p[:, su:e], w1t, xc[:, r0 * 64 + su : r0 * 64 + e],
                    start=True, stop=True,
                )
            nc.scalar.activation(
                h1c[:, r0 * 64 : r0 * 64 + w], p[:, :w], AF.Relu, bias=b1
            )
        h2c = big.tile([128, 36 * 64], bf16, tag="h2")
        for r0, nr in CT:
            w = nr * 64
            p = ps.tile([128, 1024], f32, tag="ps", name="p_h2c")
            for su in range(0, w, 512):
                e = min(su + 512, w)
                nc.tensor.matmul(
                    p[:, su:e], w2t, h1c[:, r0 * 64 + su : r0 * 64 + e],
                    start=True, stop=True,
                )
            nc.scalar.activation(
                h2c[:, r0 * 64 : r0 * 64 + w], p[:, :w], AF.Relu, bias=b2
            )
        # x3c in 66-wide zero-padded layout for the 3x3 taps
        x3c = big.tile([128, 2, 36, 66], bf16, tag="x3c")
        nc.vector.memset(x3c, 0.0)
        for cc in range(2):
            for r0, nr in CT:
                w = nr * 64
                p = ps.tile([128, 1024], f32, tag="ps", name="p_x3c")
                for su in range(0, w, 512):
                    e = min(su + 512, w)
                    nc.tensor.matmul(
                        p[:, su:e], w3t[:, cc * 128 : (cc + 1) * 128],
                        h2c[:, r0 * 64 + su : r0 * 64 + e],
                        start=True, stop=True,
                    )
                nc.scalar.activation(
                    x3c[:, cc, r0 : r0 + nr, 1:65], p[:, :w],
                    AF.Relu, bias=b3[:, cc : cc + 1],
                )
        # zero the stem-of-zero border rows (true 'SAME' pad is zero in x3)
        for cc in range(2):
            nc.vector.tensor_scalar_mul(x3c[:, cc, 0:2, :], x3c[:, cc, 0:2, :], mtop)
            nc.vector.tensor_scalar_mul(x3c[:, cc, 34:36, :], x3c[:, cc, 34:36, :], mbot)

        wb1 = load(wb1_d, [128, 2, 9, 256], tag="wb")
        y1p0 = big.tile([128, 34, 66], bf16, tag="h1")
        y1p1 = big.tile([128, 34, 66], bf16, tag="x_in")
        y1p_ = lambda ki: y1p0 if ki == 0 else y1p1
        nc.vector.memset(y1p0, 0.0)
        nc.vector.memset(y1p1, 0.0)

        # ---- S-loop / conv pieces (interleaved below) ----
        P0 = big.tile([128, NB // 2, N], bf16, tag="x3q")
        P1 = big.tile([128, NB // 2, N], bf16, tag="P1")

        def P_(nb):
            return (P0 if nb < NB // 2 else P1)[:, nb % (NB // 2), :]

        def s_block(nb):
            nsl = slice(nb * 128, (nb + 1) * 128)
            lp = singles.tile([128, 4], f32, tag="lp", bufs=4, name="lp")
            for t in range(4):
                p = ps.tile([128, 1024], f32, tag="ps", name="p_s")
                for ki in range(2):
                    for su in range(2):
                        o = t * 1024 + su * 512
                        nc.tensor.matmul(
                            p[:, su * 512 : (su + 1) * 512],
                            k_[:, ki, nsl], q[:, ki, o : o + 512],
                            start=(ki == 0), stop=(ki == 1),
                        )
                nc.scalar.activation(
                    P_(nb)[:, t * 1024 : (t + 1) * 1024], p, AF.Exp,
                    accum_out=lp[:, t : t + 1],
                )
            nc.vector.reduce_sum(out=lall[:, nb : nb + 1], in_=lp, axis=AX.X)

        def conv1_group(cc, y1row0):
            """32 y1-rows as 2 psum tiles; each LDWEIGHTS feeds 4 matmuls."""
            pA = ps.tile([128, 1024], f32, tag="ps", name="p_c1a")
            pB = ps.tile([128, 1024], f32, tag="ps", name="p_c1b")
            for kt in range(18):
                ki, tap = kt // 9, kt % 9
                dh, dw = tap // 3, tap % 3
                for ti, p in ((0, pA), (1, pB)):
                    for su in range(2):
                        r = y1row0 + ti * 16 + su * 8
                        nc.tensor.matmul(
                            p[:, su * 512 : (su + 1) * 512],
                            wb1[:, ki, tap, cc * 128 : (cc + 1) * 128],
                            x3c[:, ki, r - 1 + dh : r - 1 + dh + 8, dw : dw + 64],
                            start=(kt == 0), stop=(kt == 17),
                        )
            for ti, p in ((0, pA), (1, pB)):
                r = y1row0 + ti * 16
                nc.scalar.activation(
                    y1p_(cc)[:, r - 1 : r - 1 + 16, 1:65], p,
                    AF.Relu, bias=bb1[:, cc : cc + 1],
                )

        def conv1_tail(cc):  # y1 rows 33..34 (N=128)
            p = ps.tile([128, 1024], f32, tag="ps", name="p_c1t")
            for kt in range(18):
                ki, tap = kt // 9, kt % 9
                dh, dw = tap // 3, tap % 3
                nc.tensor.matmul(
                    p[:, 0:128],
                    wb1[:, ki, tap, cc * 128 : (cc + 1) * 128],
                    x3c[:, ki, 32 + dh : 34 + dh, dw : dw + 64],
                    start=(kt == 0), stop=(kt == 17),
                )
            nc.scalar.activation(
                y1p_(cc)[:, 32:34, 1:65], p[:, 0:128],
                AF.Relu, bias=bb1[:, cc : cc + 1],
            )

        def conv2_group(cc, orow0, wb2):
            pA = ps.tile([128, 1024], f32, tag="ps", name="p_c2a")
            pB = ps.tile([128, 1024], f32, tag="ps", name="p_c2b")
            for kt in range(18):
                ki, tap = kt // 9, kt % 9
                dh, dw = tap // 3, tap % 3
                for ti, p in ((0, pA), (1, pB)):
                    for su in range(2):
                        r = orow0 + ti * 16 + su * 8
                        nc.tensor.matmul(
                            p[:, su * 512 : (su + 1) * 512],
                            wb2[:, ki, tap, cc * 128 : (cc + 1) * 128],
                            y1p_(ki)[:, r - 2 + dh : r - 2 + dh + 8, dw : dw + 64],
                            start=(kt == 0), stop=(kt == 17),
                        )
            for ti, p in ((0, pA), (1, pB)):
                r = orow0 + ti * 16
                st = big.tile([128, 1024], f32, tag=("h2" if ti else "x3c"), name="st_c")
                nc.vector.tensor_scalar_add(st, p, bb2[:, cc : cc + 1])
                nc.sync.dma_start(
                    out=oc_d[cc * 128 : (cc + 1) * 128, (r - 2) * 64 : (r - 2) * 64 + 1024],
                    in_=st,
                )

        # ---- interleave: S blocks are ScalarE(exp)-paced; conv groups keep
        #      TensorE busy meanwhile ----
        s_block(0)
        s_block(1)
        conv1_group(0, 1)
        s_block(2)
        s_block(3)
        conv1_group(1, 1)
        s_block(4)
        s_block(5)
        conv1_tail(0)
        conv1_tail(1)
        s_block(6)
        for cc in range(2):
            nc.vector.tensor_scalar_mul(y1p_(cc)[:, 0, :], y1p_(cc)[:, 0, :], mtop)
            nc.vector.tensor_scalar_mul(y1p_(cc)[:, 33, :], y1p_(cc)[:, 33, :], mbot)
        wb2 = load(wb2_d, [128, 2, 9, 256], tag="wb")
        s_block(7)
        s_block(8)
        conv2_group(0, 2, wb2)
        s_block(9)
        s_block(10)
        conv2_group(1, 2, wb2)
        s_block(11)
        s_block(12)
        s_block(13)
        s_block(14)
        s_block(15)

        # ---- fold 1/l into vT (softmax normalizer on the contracted axis) ----
        nc.vector.reciprocal(rl, lall)
        for nb in range(NB):
            nc.vector.tensor_scalar_mul(vT[:, nb, :], vT[:, nb, :], rl[:, nb : nb + 1])

        # ---- attn_out partial = (v/l) @ P; all 4 psum slots per cc,
        #      weight-stationary over nb (each LDWEIGHTS feeds 8 matmuls) ----
        for cc in range(2):
            pt = [ps.tile([128, 1024], f32, tag="ps", name=f"p_at{t}") for t in range(4)]
            for nb in range(NB):
                for t in range(4):
                    for su in range(2):
                        o = t * 1024 + su * 512
                        nc.tensor.matmul(
                            pt[t][:, su * 512 : (su + 1) * 512],
                            vT[:, nb, cc * 128 : (cc + 1) * 128],
                            P_(nb)[:, o : o + 512],
                            start=(nb == 0), stop=(nb == NB - 1),
                        )
            for t in range(4):
                st = big.tile([128, 1024], f32, tag=("h2" if t % 2 else "x3c"), name="st_a")
                nc.vector.tensor_copy(st, pt[t])
                nc.sync.dma_start(
                    out=oa_d[cc * 128 : (cc + 1) * 128, t * 1024 : (t + 1) * 1024],
                    in_=st,
                )

    nc.compile()
    return nc


def _get_nc():
    if "nc" not in _CACHE:
        _CACHE["nc"] = _build_nc()
    return _CACHE["nc"]


def _make_in_maps(x, w1, b1, w2, b2, w3, b3, wb1, bb1, wb2, bb2,
                  wq, bq, wk, bk, wv, bv):
    bfc = lambda a: np.ascontiguousarray(np.asarray(a, np.float32).astype(ml_dtypes.bfloat16))
    f32c = lambda a: np.ascontiguousarray(np.asarray(a, np.float32))

    def qkv_t(w):  # [O, CI] -> lhsT/rhs chunks [128, 2, 256]
        return bfc(np.asarray(w, np.float32).T.reshape(2, 128, 256).transpose(1, 0, 2))

    def conv_t(wb):  # [O, I, 3, 3] -> [128 kip, 2 ki, 9 tap, 256 o]
        a = np.asarray(wb, np.float32).transpose(1, 0, 2, 3)  # [I, O, 3, 3]
        a = a.reshape(2, 128, 256, 9)                          # [ki, kip, o, tap]
        return bfc(a.transpose(1, 0, 3, 2))                    # [kip, ki, tap, o]

    def bias2(b):  # [256] -> [128, 2] (col cc = chunk cc)
        return f32c(np.asarray(b, np.float32).reshape(2, 128).T)

    common = {
        "w1t": bfc(np.asarray(w1).T), "b1": f32c(np.asarray(b1).reshape(64, 1)),
        "w2t": bfc(np.asarray(w2).T), "b2": f32c(np.asarray(b2).reshape(128, 1)),
        "w3t": bfc(np.asarray(w3).T), "b3": bias2(b3),
        "wqt": qkv_t(wq), "bq": bias2(bq),
        "wkt": qkv_t(wk), "bk": bias2(bk),
        "wvt": qkv_t(wv), "bv": bfc(np.asarray(bv).reshape(1, 256)),
        "wb1": conv_t(wb1), "bb1": bias2(bb1),
        "wb2": conv_t(wb2), "bb2": bias2(bb2),
    }

    xf = np.asarray(x, np.float32).reshape(B, 3, N)
    in_maps = []
    for core in range(8):
        b, h = core // 2, core % 2
        xq = bfc(np.roll(xf[b], -NH * h, axis=1))
        # conv window: global rows [32h-2, 32h+34), zero outside the image
        xi = xf[b].reshape(3, H, W)
        xc = np.zeros((3, 36, W), np.float32)
        g0 = 32 * h - 2
        lo, hi = max(0, g0), min(H, g0 + 36)
        xc[:, lo - g0 : hi - g0, :] = xi[:, lo:hi, :]
        m = np.full((128, 1), 1.0, np.float32)
        z = np.full((128, 1), 0.0, np.float32)
        in_maps.append(dict(
            common,
            xq=xq,
            xc=bfc(xc.reshape(3, 36 * 64)),
            mtop=(z if h == 0 else m),
            mbot=(m if h == 0 else z),
        ))
    return in_maps


def _gather(results, alpha, beta):
    a, bt = float(alpha), float(beta)
    out = np.empty((B, C, H, W), np.float32)
    for b in range(B):
        r0, r1 = results[2 * b], results[2 * b + 1]
        attn = r0["out_attn"] + np.roll(r1["out_attn"], NH, axis=1)
        conv = np.concatenate(
            [r0["out_conv"].reshape(C, 32, W), r1["out_conv"].reshape(C, 32, W)],
            axis=1,
        )
        out[b] = a * conv + bt * attn.reshape(C, H, W)
    return out


def _run(inputs, trace=False, **kw):
    from concourse import bass_utils

    nc = _get_nc()
    in_maps = _make_in_maps(
        inputs["x"], inputs["w1"], inputs["b1"], inputs["w2"], inputs["b2"],
        inputs["w3"], inputs["b3"], inputs["wb1"], inputs["bb1"],
        inputs["wb2"], inputs["bb2"], inputs["wq"], inputs["bq"],
        inputs["wk"], inputs["bk"], inputs["wv"], inputs["bv"],
    )
    res = bass_utils.run_bass_kernel_spmd(
        nc, in_maps, core_ids=list(range(8)), trace=trace, **kw
    )
    return _gather(res.results, inputs["alpha"], inputs["beta"]), res


def kernel(**inputs):
    out, _ = _run(inputs, trace=False)
    return out


# revision 12
# speedup vs baseline: 1.1898x; 1.0601x over previous
"""Trainium2 Bass kernel for nn_AttCM: 1x1-conv stem -> (two 3x3 convs) +
(single-head spatial attention), alpha/beta combined.

Sharding: 8 cores = 4 samples x 2 halves of the attention key axis (n).
Each core computes the full stem + q for its sample (cheap), its n-half of
S = k^T q with full softmax rows (softmax axis is m, fully local), a partial
attn_out = (v/l) @ exp(S) (host adds the two partials), and half of the 3x3
conv branch rows. No cross-core communication; the host applies
alpha*conv + beta*attn and the inverse of the per-core pixel roll.

SPMD trick: all 8 cores run one graph. Per-core behavior comes from data:
  - xq is the sample pixel-rolled by -2048*h so the core's k/v half is always
    columns [0, 2048) of its local x3; the attention output columns are rolled
    back on the host.
  - xc is a 36-row window of the sample (host zero-padded at image borders)
    so the conv branch always computes local output rows 2..33.
  - mtop/mbot (0.0 or 1.0 per core) zero the stem-of-zero padding rows that
    a true conv 'SAME' zero-pad requires.

All matmul inputs are bf16 (fp32 PSUM accumulation); measured rel_l2 vs the
fp32 reference ~2e-3.

Schedule notes: the S loop is ScalarE-bound (exp of 8.4M elements/core), so
the 3x3 conv matmul groups are interleaved between S blocks to keep TensorE
busy while ScalarE drains exp; PSUM is managed as 4 slots of 2 banks each
(2 for S ping-pong, 2 for the interleaved conv groups).
"""

import numpy as np
import ml_dtypes

_CACHE = {}

B, C, H, W = 4, 256, 64, 64
N = H * W            # 4096 pixels
NH = N // 2          # per-core attention key half
NB = 16              # n-blocks of 128 rows per core


def _build_nc():
    from contextlib import ExitStack

    import concourse.mybir as mybir
    import concourse.tile as tile
    from concourse import bacc

    f32 = mybir.dt.float32
    bf16 = mybir.dt.bfloat16
    AF = mybir.ActivationFunctionType
    AX = mybir.AxisListType

    nc = bacc.Bacc("TRN2", target_bir_lowering=False, debug=False)

    def din(name, shape, dt=bf16):
        return nc.dram_tensor(name, shape, dt, kind="ExternalInput").ap()

    xq_d = din("xq", [3, N])
    xc_d = din("xc", [3, 36 * 64])
    wsb_d = din("wsb", [128, 2240])
    fsb_d = din("fsb", [128, 14], f32)
    wb1_d = din("wb1", [128, 2, 9, 256])
    wb2_d = din("wb2", [128, 2, 9, 256])

    oa_d = nc.dram_tensor("out_attn", [C, N], f32, kind="ExternalOutput").ap()
    oc_d = nc.dram_tensor("out_conv", [C, 32 * 64], f32, kind="ExternalOutput").ap()

    with tile.TileContext(nc) as tc, ExitStack() as ctx:
        singles = ctx.enter_context(tc.tile_pool(name="singles", bufs=1))
        ps = ctx.enter_context(tc.tile_pool(name="ps", bufs=4, space="PSUM"))
        big = ctx.enter_context(tc.tile_pool(name="big", bufs=1))

        def load(d, shape, dt=bf16, tag=None):
            nm = d.tensor.name + "_sb"
            t = (singles.tile(shape, dt, tag=tag, name=nm) if tag
                 else singles.tile(shape, dt, name=nm))
            nc.sync.dma_start(out=t, in_=d)
            return t

        wsb = singles.tile([128, 2240], bf16, name="wsb")
        fsb = singles.tile([128, 14], f32, name="fsb")
        nc.gpsimd.dma_start(out=wsb, in_=wsb_d)
        nc.gpsimd.dma_start(out=fsb, in_=fsb_d)
        w1t = wsb[0:3, 0:64]
        w2t = wsb[0:64, 64:192]
        w3t = wsb[:, 192:448]
        wqt = wsb[:, 448:960].rearrange("p (a b) -> p a b", a=2)
        wkt = wsb[:, 960:1472].rearrange("p (a b) -> p a b", a=2)
        wvt = wsb[:, 1472:1984].rearrange("p (a b) -> p a b", a=2)
        bv = wsb[0:1, 1984:2240]
        b1 = fsb[0:64, 0:1]
        b2 = fsb[:, 1:2]
        b3 = fsb[:, 2:4]
        bq = fsb[:, 4:6]
        bk = fsb[:, 6:8]
        bb1 = fsb[:, 8:10]
        bb2 = fsb[:, 10:12]
        mtop = fsb[:, 12:13]
        mbot = fsb[:, 13:14]
        ones = singles.tile([1, 128], bf16)
        nc.vector.memset(ones, 1.0)
        lall = singles.tile([128, NB], f32)
        rl = singles.tile([128, NB], f32)

        # ---- stem on the rolled full sample (feeds q, k, v) ----
        xq = big.tile([3, N], bf16, tag="x_in")
        nc.sync.dma_start(out=xq, in_=xq_d)
        h1 = big.tile([64, N], bf16, tag="h1")
        for t in range(4):
            p = ps.tile([64, 1024], f32, tag="ps", name="p_h1")
            for su in range(2):
                nc.tensor.matmul(
                    p[:, su * 512 : (su + 1) * 512], w1t,
                    xq[:, t * 1024 + su * 512 : t * 1024 + (su + 1) * 512],
                    start=True, stop=True,
                )
            if t % 2 == 0:
                nc.scalar.activation(h1[:, t * 1024 : (t + 1) * 1024], p, AF.Relu, bias=b1)
            else:
                nc.vector.tensor_scalar(h1[:, t * 1024 : (t + 1) * 1024], p, b1, 0.0,
                                        op0=mybir.AluOpType.add, op1=mybir.AluOpType.max)
        h2 = big.tile([128, N], bf16, tag="h2")
        for t in range(4):
            p = ps.tile([128, 1024], f32, tag="ps", name="p_h2")
            for su in range(2):
                nc.tensor.matmul(
                    p[:, su * 512 : (su + 1) * 512], w2t,
                    h1[:, t * 1024 + su * 512 : t * 1024 + (su + 1) * 512],
                    start=True, stop=True,
                )
            if t % 2 == 0:
                nc.scalar.activation(h2[:, t * 1024 : (t + 1) * 1024], p, AF.Relu, bias=b2)
            else:
                nc.vector.tensor_scalar(h2[:, t * 1024 : (t + 1) * 1024], p, b2, 0.0,
                                        op0=mybir.AluOpType.add, op1=mybir.AluOpType.max)
        x3q = big.tile([128, 2, N], bf16, tag="x3q")
        for cc in range(2):
            for t in range(4):
                p = ps.tile([128, 1024], f32, tag="ps", name="p_x3q")
                for su in range(2):
                    nc.tensor.matmul(
                        p[:, su * 512 : (su + 1) * 512],
                        w3t[:, cc * 128 : (cc + 1) * 128],
                        h2[:, t * 1024 + su * 512 : t * 1024 + (su + 1) * 512],
                        start=True, stop=True,
                    )
                if t % 2 == 0:
                    nc.scalar.activation(
                        x3q[:, cc, t * 1024 : (t + 1) * 1024], p,
                        AF.Relu, bias=b3[:, cc : cc + 1],
                    )
                else:
                    nc.vector.tensor_scalar(
                        x3q[:, cc, t * 1024 : (t + 1) * 1024], p,
                        b3[:, cc : cc + 1], 0.0,
                        op0=mybir.AluOpType.add, op1=mybir.AluOpType.max,
                    )

        # ---- q (full m), k (local n half), vT (local n half, transposed) ----
        q = big.tile([128, 2, N], bf16, tag="q")
        for cc in range(2):
            for t in range(4):
                p = ps.tile([128, 1024], f32, tag="ps", name="p_q")
                for ki in range(2):
                    for su in range(2):
                        nc.tensor.matmul(
                            p[:, su * 512 : (su + 1) * 512],
                            wqt[:, ki, cc * 128 : (cc + 1) * 128],
                            x3q[:, ki, t * 1024 + su * 512 : t * 1024 + (su + 1) * 512],
                            start=(ki == 0), stop=(ki == 1),
                        )
                nc.vector.tensor_scalar_add(
                    q[:, cc, t * 1024 : (t + 1) * 1024], p, bq[:, cc : cc + 1]
                )
        k_ = big.tile([128, 2, NH], bf16, tag="k")
        for cc in range(2):
            for t in range(2):
                p = ps.tile([128, 1024], f32, tag="ps", name="p_k")
                for ki in range(2):
                    for su in range(2):
                        nc.tensor.matmul(
                            p[:, su * 512 : (su + 1) * 512],
                            wkt[:, ki, cc * 128 : (cc + 1) * 128],
                            x3q[:, ki, t * 1024 + su * 512 : t * 1024 + (su + 1) * 512],
                            start=(ki == 0), stop=(ki == 1),
                        )
                nc.vector.tensor_scalar_add(
                    k_[:, cc, t * 1024 : (t + 1) * 1024], p, bk[:, cc : cc + 1]
                )
        # vT[n, c] = sum_ci x3[ci, n] WvT[ci, c] + bv[c]  (bias via K=1 matmul)
        vT = big.tile([128, NB, 256], bf16, tag="vT")
        for g in range(4):
            p = ps.tile([128, 1024], f32, tag="ps", name="p_vT")
            for j in range(4):
                nb = g * 4 + j
                nsl = slice(nb * 128, (nb + 1) * 128)
                o = slice(j * 256, (j + 1) * 256)
                nc.tensor.matmul(p[:, o], x3q[:, 0, nsl], wvt[:, 0, :], start=True, stop=False)
                nc.tensor.matmul(p[:, o], x3q[:, 1, nsl], wvt[:, 1, :], start=False, stop=False)
                nc.tensor.matmul(p[:, o], ones, bv, start=False, stop=True)
            nc.vector.tensor_copy(vT[:, g * 4 : (g + 1) * 4, :], p)

        # ---- stem on the conv window (local rows 0..35 incl. border pads) ----
        xc = big.tile([3, 36 * 64], bf16, tag="x_in")
        nc.sync.dma_start(out=xc, in_=xc_d)
        CT = [(0, 16), (16, 16), (32, 4)]  # (row0, nrows)
        h1c = big.tile([64, 36 * 64], bf16, tag="h1")
        for r0, nr in CT:
            w = nr * 64
            p = ps.tile([64, 1024], f32, tag="ps", name="p_h1c")
            for su in range(0, w, 512):
                e = min(su + 512, w)
                nc.tensor.matmul(
                    p[:, su:e], w1t, xc[:, r0 * 64 + su : r0 * 64 + e],
                    start=True, stop=True,
                )
            nc.scalar.activation(
                h1c[:, r0 * 64 : r0 * 64 + w], p[:, :w], AF.Relu, bias=b1
            )
        h2c = big.tile([128, 36 * 64], bf16, tag="h2")
        for r0, nr in CT:
            w = nr * 64
            p = ps.tile([128, 1024], f32, tag="ps", name="p_h2c")
            for su in range(0, w, 512):
                e = min(su + 512, w)
                nc.tensor.matmul(
                    p[:, su:e], w2t, h1c[:, r0 * 64 + su : r0 * 64 + e],
                    start=True, stop=True,
                )
            nc.scalar.activation(
                h2c[:, r0 * 64 : r0 * 64 + w], p[:, :w], AF.Relu, bias=b2
            )
        # x3c in 66-wide zero-padded layout for the 3x3 taps
        x3c = big.tile([128, 2, 36, 66], bf16, tag="x3c")
        nc.vector.memset(x3c, 0.0)
        for cc in range(2):
            for r0, nr in CT:
                w = nr * 64
                p = ps.tile([128, 1024], f32, tag="ps", name="p_x3c")
                for su in range(0, w, 512):
                    e = min(su + 512, w)
                    nc.tensor.matmul(
                        p[:, su:e], w3t[:, cc * 128 : (cc + 1) * 128],
                        h2c[:, r0 * 64 + su : r0 * 64 + e],
                        start=True, stop=True,
                    )
                nc.scalar.activation(
                    x3c[:, cc, r0 : r0 + nr, 1:65], p[:, :w],
                    AF.Relu, bias=b3[:, cc : cc + 1],
                )
        # zero the stem-of-zero border rows (true 'SAME' pad is zero in x3)
        for cc in range(2):
            nc.vector.tensor_scalar_mul(x3c[:, cc, 0:2, :], x3c[:, cc, 0:2, :], mtop)
            nc.vector.tensor_scalar_mul(x3c[:, cc, 34:36, :], x3c[:, cc, 34:36, :], mbot)

        wb1 = singles.tile([128, 2, 9, 256], bf16, tag="wb", name="wb1_sb")
        nc.scalar.dma_start(out=wb1, in_=wb1_d)
        y1p0 = big.tile([128, 34, 66], bf16, tag="h1")
        y1p1 = big.tile([128, 34, 66], bf16, tag="x_in")
        y1p_ = lambda ki: y1p0 if ki == 0 else y1p1
        nc.vector.memset(y1p0, 0.0)
        nc.vector.memset(y1p1, 0.0)

        # ---- S-loop / conv pieces (interleaved below) ----
        P0 = big.tile([128, NB // 2, N], bf16, tag="x3q")
        P1 = big.tile([128, NB // 2, N], bf16, tag="P1")

        def P_(nb):
            return (P0 if nb < NB // 2 else P1)[:, nb % (NB // 2), :]

        def s_block(nb):
            nsl = slice(nb * 128, (nb + 1) * 128)
            lp = singles.tile([128, 4], f32, tag="lp", bufs=4, name="lp")
            for t in range(4):
                p = ps.tile([128, 1024], f32, tag="ps", name="p_s")
                for ki in range(2):
                    for su in range(2):
                        o = t * 1024 + su * 512
                        nc.tensor.matmul(
                            p[:, su * 512 : (su + 1) * 512],
                            k_[:, ki, nsl], q[:, ki, o : o + 512],
                            start=(ki == 0), stop=(ki == 1),
                        )
                nc.scalar.activation(
                    P_(nb)[:, t * 1024 : (t + 1) * 1024], p, AF.Exp,
                    accum_out=lp[:, t : t + 1],
                )
            nc.vector.reduce_sum(out=lall[:, nb : nb + 1], in_=lp, axis=AX.X)
            nc.vector.reciprocal(rl[:, nb : nb + 1], lall[:, nb : nb + 1])
            nc.vector.tensor_scalar_mul(vT[:, nb, :], vT[:, nb, :], rl[:, nb : nb + 1])

        def conv1_group(cc, y1row0):
            """32 y1-rows as 2 psum tiles; each LDWEIGHTS feeds 4 matmuls."""
            pA = ps.tile([128, 1024], f32, tag="ps", name="p_c1a")
            pB = ps.tile([128, 1024], f32, tag="ps", name="p_c1b")
            for kt in range(18):
                ki, tap = kt // 9, kt % 9
                dh, dw = tap // 3, tap % 3
                for ti, p in ((0, pA), (1, pB)):
                    for su in range(2):
                        r = y1row0 + ti * 16 + su * 8
                        nc.tensor.matmul(
                            p[:, su * 512 : (su + 1) * 512],
                            wb1[:, ki, tap, cc * 128 : (cc + 1) * 128],
                            x3c[:, ki, r - 1 + dh : r - 1 + dh + 8, dw : dw + 64],
                            start=(kt == 0), stop=(kt == 17),
                        )
            for ti, p in ((0, pA), (1, pB)):
                r = y1row0 + ti * 16
                nc.scalar.activation(
                    y1p_(cc)[:, r - 1 : r - 1 + 16, 1:65], p,
                    AF.Relu, bias=bb1[:, cc : cc + 1],
                )

        def conv1_tail(cc):  # y1 rows 33..34 (N=128)
            p = ps.tile([128, 1024], f32, tag="ps", name="p_c1t")
            for kt in range(18):
                ki, tap = kt // 9, kt % 9
                dh, dw = tap // 3, tap % 3
                nc.tensor.matmul(
                    p[:, 0:128],
                    wb1[:, ki, tap, cc * 128 : (cc + 1) * 128],
                    x3c[:, ki, 32 + dh : 34 + dh, dw : dw + 64],
                    start=(kt == 0), stop=(kt == 17),
                )
            nc.scalar.activation(
                y1p_(cc)[:, 32:34, 1:65], p[:, 0:128],
                AF.Relu, bias=bb1[:, cc : cc + 1],
            )

        def conv2_group(cc, orow0, wb2):
            pA = ps.tile([128, 1024], f32, tag="ps", name="p_c2a")
            pB = ps.tile([128, 1024], f32, tag="ps", name="p_c2b")
            for kt in range(18):
                ki, tap = kt // 9, kt % 9
                dh, dw = tap // 3, tap % 3
                for ti, p in ((0, pA), (1, pB)):
                    for su in range(2):
                        r = orow0 + ti * 16 + su * 8
                        nc.tensor.matmul(
                            p[:, su * 512 : (su + 1) * 512],
                            wb2[:, ki, tap, cc * 128 : (cc + 1) * 128],
                            y1p_(ki)[:, r - 2 + dh : r - 2 + dh + 8, dw : dw + 64],
                            start=(kt == 0), stop=(kt == 17),
                        )
            for ti, p in ((0, pA), (1, pB)):
                r = orow0 + ti * 16
                st = big.tile([128, 1024], f32, tag=("h2" if ti else "x3c"), name="st_c")
                nc.vector.tensor_scalar_add(st, p, bb2[:, cc : cc + 1])
                nc.sync.dma_start(
                    out=oc_d[cc * 128 : (cc + 1) * 128, (r - 2) * 64 : (r - 2) * 64 + 1024],
                    in_=st,
                )

        # ---- interleave: S blocks are ScalarE(exp)-paced; conv groups keep
        #      TensorE busy meanwhile ----
        s_block(0)
        s_block(1)
        conv1_group(0, 1)
        s_block(2)
        s_block(3)
        conv1_group(1, 1)
        s_block(4)
        s_block(5)
        conv1_tail(0)
        conv1_tail(1)
        s_block(6)
        for cc in range(2):
            nc.vector.tensor_scalar_mul(y1p_(cc)[:, 0, :], y1p_(cc)[:, 0, :], mtop)
            nc.vector.tensor_scalar_mul(y1p_(cc)[:, 33, :], y1p_(cc)[:, 33, :], mbot)
        wb2 = singles.tile([128, 2, 9, 256], bf16, tag="wb", name="wb2_sb")
        nc.scalar.dma_start(out=wb2, in_=wb2_d)
        s_block(7)
        s_block(8)
        conv2_group(0, 2, wb2)
        s_block(9)
        s_block(10)
        conv2_group(1, 2, wb2)
        s_block(11)
        s_block(12)
        s_block(13)
        s_block(14)
        s_block(15)

        # ---- attn_out partial = (v/l) @ P; all 4 psum slots per cc,
        #      weight-stationary over nb (each LDWEIGHTS feeds 8 matmuls) ----
        for cc in range(2):
            pt = [ps.tile([128, 1024], f32, tag="ps", name=f"p_at{t}") for t in range(4)]
            for nb in range(NB):
                for t in range(4):
                    for su in range(2):
                        o = t * 1024 + su * 512
                        nc.tensor.matmul(
                            pt[t][:, su * 512 : (su + 1) * 512],
                            vT[:, nb, cc * 128 : (cc + 1) * 128],
                            P_(nb)[:, o : o + 512],
                            start=(nb == 0), stop=(nb == NB - 1),
                        )
            for t in range(4):
                st = big.tile([128, 1024], f32, tag=("h2" if t % 2 else "x3c"), name="st_a")
                nc.vector.tensor_copy(st, pt[t])
                nc.sync.dma_start(
                    out=oa_d[cc * 128 : (cc + 1) * 128, t * 1024 : (t + 1) * 1024],
                    in_=st,
                )

    nc.compile()
    return nc


def _get_nc():
    if "nc" not in _CACHE:
        _CACHE["nc"] = _build_nc()
    return _CACHE["nc"]


def _make_in_maps(x, w1, b1, w2, b2, w3, b3, wb1, bb1, wb2, bb2,
                  wq, bq, wk, bk, wv, bv):
    bfc = lambda a: np.ascontiguousarray(np.asarray(a, np.float32).astype(ml_dtypes.bfloat16))
    f32c = lambda a: np.ascontiguousarray(np.asarray(a, np.float32))

    def qkv_t(w):  # [O, CI] -> lhsT/rhs chunks [128, 2, 256]
        return bfc(np.asarray(w, np.float32).T.reshape(2, 128, 256).transpose(1, 0, 2))

    def conv_t(wb):  # [O, I, 3, 3] -> [128 kip, 2 ki, 9 tap, 256 o]
        a = np.asarray(wb, np.float32).transpose(1, 0, 2, 3)  # [I, O, 3, 3]
        a = a.reshape(2, 128, 256, 9)                          # [ki, kip, o, tap]
        return bfc(a.transpose(1, 0, 3, 2))                    # [kip, ki, tap, o]

    def bias2(b):  # [256] -> [128, 2] (col cc = chunk cc)
        return f32c(np.asarray(b, np.float32).reshape(2, 128).T)

    wsb = np.zeros((128, 2240), np.float32)
    wsb[0:3, 0:64] = np.asarray(w1).T
    wsb[0:64, 64:192] = np.asarray(w2).T
    wsb[:, 192:448] = np.asarray(w3).T
    wsb[:, 448:960] = qkv_t(wq).astype(np.float32).reshape(128, 512)
    wsb[:, 960:1472] = qkv_t(wk).astype(np.float32).reshape(128, 512)
    wsb[:, 1472:1984] = qkv_t(wv).astype(np.float32).reshape(128, 512)
    wsb[0, 1984:2240] = np.asarray(bv)
    fsb = np.zeros((128, 14), np.float32)
    fsb[0:64, 0] = np.asarray(b1)
    fsb[:, 1] = np.asarray(b2)
    fsb[:, 2:4] = bias2(b3)
    fsb[:, 4:6] = bias2(bq)
    fsb[:, 6:8] = bias2(bk)
    fsb[:, 8:10] = bias2(bb1)
    fsb[:, 10:12] = bias2(bb2)
    # fsb[:, 12:14] = per-core mtop/mbot, filled below
    common = {
        "wsb": bfc(wsb),
        "wb1": conv_t(wb1),
        "wb2": conv_t(wb2),
    }

    xf = np.asarray(x, np.float32).reshape(B, 3, N)
    in_maps = []
    for core in range(8):
        b, h = core // 2, core % 2
        xq = bfc(np.roll(xf[b], -NH * h, axis=1))
        # conv window: global rows [32h-2, 32h+34), zero outside the image
        xi = xf[b].reshape(3, H, W)
        xc = np.zeros((3, 36, W), np.float32)
        g0 = 32 * h - 2
        lo, hi = max(0, g0), min(H, g0 + 36)
        xc[:, lo - g0 : hi - g0, :] = xi[:, lo:hi, :]
        fc = fsb.copy()
        fc[:, 12] = 0.0 if h == 0 else 1.0
        fc[:, 13] = 1.0 if h == 0 else 0.0
        in_maps.append(dict(
            common,
            xq=xq,
            xc=bfc(xc.reshape(3, 36 * 64)),
            fsb=f32c(fc),
        ))
    return in_maps


def _gather(results, alpha, beta):
    a, bt = float(alpha), float(beta)
    out = np.empty((B, C, H, W), np.float32)
    for b in range(B):
        r0, r1 = results[2 * b], results[2 * b + 1]
        attn = r0["out_attn"] + np.roll(r1["out_attn"], NH, axis=1)
        conv = np.concatenate(
            [r0["out_conv"].reshape(C, 32, W), r1["out_conv"].reshape(C, 32, W)],
            axis=1,
        )
        out[b] = a * conv + bt * attn.reshape(C, H, W)
    return out


def _run(inputs, trace=False, **kw):
    from concourse import bass_utils

    nc = _get_nc()
    in_maps = _make_in_maps(
        inputs["x"], inputs["w1"], inputs["b1"], inputs["w2"], inputs["b2"],
        inputs["w3"], inputs["b3"], inputs["wb1"], inputs["bb1"],
        inputs["wb2"], inputs["bb2"], inputs["wq"], inputs["bq"],
        inputs["wk"], inputs["bk"], inputs["wv"], inputs["bv"],
    )
    res = bass_utils.run_bass_kernel_spmd(
        nc, in_maps, core_ids=list(range(8)), trace=trace, **kw
    )
    return _gather(res.results, inputs["alpha"], inputs["beta"]), res


def kernel(**inputs):
    out, _ = _run(inputs, trace=False)
    return out


# revision 13
# speedup vs baseline: 1.2111x; 1.0179x over previous
"""Trainium2 Bass kernel for nn_AttCM: 1x1-conv stem -> (two 3x3 convs) +
(single-head spatial attention), alpha/beta combined.

Sharding: 8 cores = 4 samples x 2 halves of the attention key axis (n).
Each core computes the full stem + q for its sample (cheap), its n-half of
S = k^T q with full softmax rows (softmax axis is m, fully local), a partial
attn_out = (v/l) @ exp(S) (host adds the two partials), and half of the 3x3
conv branch rows. No cross-core communication; the host applies
alpha*conv + beta*attn and the inverse of the per-core pixel roll.

SPMD trick: all 8 cores run one graph. Per-core behavior comes from data:
  - xq is the sample pixel-rolled by -2048*h so the core's k/v half is always
    columns [0, 2048) of its local x3; the attention output columns are rolled
    back on the host.
  - xc is a 36-row window of the sample (host zero-padded at image borders)
    so the conv branch always computes local output rows 2..33.
  - mtop/mbot (0.0 or 1.0 per core) zero the stem-of-zero padding rows that
    a true conv 'SAME' zero-pad requires.

All matmul inputs are bf16 (fp32 PSUM accumulation); measured rel_l2 vs the
fp32 reference ~2e-3.

Schedule notes: the S loop is ScalarE-bound (exp of 8.4M elements/core), so
the 3x3 conv matmul groups are interleaved between S blocks to keep TensorE
busy while ScalarE drains exp; PSUM is managed as 4 slots of 2 banks each
(2 for S ping-pong, 2 for the interleaved conv groups).
"""

import numpy as np
import ml_dtypes

_CACHE = {}

B, C, H, W = 4, 256, 64, 64
N = H * W            # 4096 pixels
NH = N // 2          # per-core attention key half
NB = 16              # n-blocks of 128 rows per core


def _build_nc():
    from contextlib import ExitStack

    import concourse.mybir as mybir
    import concourse.tile as tile
    from concourse import bacc

    f32 = mybir.dt.float32
    bf16 = mybir.dt.bfloat16
    AF = mybir.ActivationFunctionType
    AX = mybir.AxisListType

    nc = bacc.Bacc("TRN2", target_bir_lowering=False, debug=False)

    def din(name, shape, dt=bf16):
        return nc.dram_tensor(name, shape, dt, kind="ExternalInput").ap()

    xq_d = din("xq", [3, N])
    wsb_d = din("wsb", [128, 2240])
    fsb_d = din("fsb", [128, 14], f32)
    wb1_d = din("wb1", [128, 2, 9, 256])
    wb2_d = din("wb2", [128, 2, 9, 256])

    oa_d = nc.dram_tensor("out_attn", [C, N], f32, kind="ExternalOutput").ap()
    oc_d = nc.dram_tensor("out_conv", [C, 32 * 64], f32, kind="ExternalOutput").ap()

    with tile.TileContext(nc) as tc, ExitStack() as ctx:
        singles = ctx.enter_context(tc.tile_pool(name="singles", bufs=1))
        ps = ctx.enter_context(tc.tile_pool(name="ps", bufs=4, space="PSUM"))
        big = ctx.enter_context(tc.tile_pool(name="big", bufs=1))

        def load(d, shape, dt=bf16, tag=None):
            nm = d.tensor.name + "_sb"
            t = (singles.tile(shape, dt, tag=tag, name=nm) if tag
                 else singles.tile(shape, dt, name=nm))
            nc.sync.dma_start(out=t, in_=d)
            return t

        wsb = singles.tile([128, 2240], bf16, name="wsb")
        fsb = singles.tile([128, 14], f32, name="fsb")
        nc.gpsimd.dma_start(out=wsb, in_=wsb_d)
        nc.gpsimd.dma_start(out=fsb, in_=fsb_d)
        w1t = wsb[0:3, 0:64]
        w2t = wsb[0:64, 64:192]
        w3t = wsb[:, 192:448]
        wqt = wsb[:, 448:960].rearrange("p (a b) -> p a b", a=2)
        wkt = wsb[:, 960:1472].rearrange("p (a b) -> p a b", a=2)
        wvt = wsb[:, 1472:1984].rearrange("p (a b) -> p a b", a=2)
        bv = wsb[0:1, 1984:2240]
        b1 = fsb[0:64, 0:1]
        b2 = fsb[:, 1:2]
        b3 = fsb[:, 2:4]
        bq = fsb[:, 4:6]
        bk = fsb[:, 6:8]
        bb1 = fsb[:, 8:10]
        bb2 = fsb[:, 10:12]
        mtop = fsb[:, 12:13]
        mbot = fsb[:, 13:14]
        ones = singles.tile([1, 128], bf16)
        nc.vector.memset(ones, 1.0)
        lall = singles.tile([128, NB], f32)
        rl = singles.tile([128, NB], f32)

        # ---- stem on the rolled full sample (feeds q, k, v) ----
        xq = big.tile([3, N], bf16, tag="x_in")
        nc.sync.dma_start(out=xq, in_=xq_d)
        h1 = big.tile([64, N], bf16, tag="h1")
        for t in range(4):
            p = ps.tile([64, 1024], f32, tag="ps", name="p_h1")
            for su in range(2):
                nc.tensor.matmul(
                    p[:, su * 512 : (su + 1) * 512], w1t,
                    xq[:, t * 1024 + su * 512 : t * 1024 + (su + 1) * 512],
                    start=True, stop=True,
                )
            if t % 2 == 0:
                nc.scalar.activation(h1[:, t * 1024 : (t + 1) * 1024], p, AF.Relu, bias=b1)
            else:
                nc.vector.tensor_scalar(h1[:, t * 1024 : (t + 1) * 1024], p, b1, 0.0,
                                        op0=mybir.AluOpType.add, op1=mybir.AluOpType.max)
        h2 = big.tile([128, N], bf16, tag="h2")
        for t in range(4):
            p = ps.tile([128, 1024], f32, tag="ps", name="p_h2")
            for su in range(2):
                nc.tensor.matmul(
                    p[:, su * 512 : (su + 1) * 512], w2t,
                    h1[:, t * 1024 + su * 512 : t * 1024 + (su + 1) * 512],
                    start=True, stop=True,
                )
            if t % 2 == 0:
                nc.scalar.activation(h2[:, t * 1024 : (t + 1) * 1024], p, AF.Relu, bias=b2)
            else:
                nc.vector.tensor_scalar(h2[:, t * 1024 : (t + 1) * 1024], p, b2, 0.0,
                                        op0=mybir.AluOpType.add, op1=mybir.AluOpType.max)
        x3q = big.tile([128, 2, N], bf16, tag="x3q")
        for cc in range(2):
            for t in range(4):
                p = ps.tile([128, 1024], f32, tag="ps", name="p_x3q")
                for su in range(2):
                    nc.tensor.matmul(
                        p[:, su * 512 : (su + 1) * 512],
                        w3t[:, cc * 128 : (cc + 1) * 128],
                        h2[:, t * 1024 + su * 512 : t * 1024 + (su + 1) * 512],
                        start=True, stop=True,
                    )
                if t % 2 == 0:
                    nc.scalar.activation(
                        x3q[:, cc, t * 1024 : (t + 1) * 1024], p,
                        AF.Relu, bias=b3[:, cc : cc + 1],
                    )
                else:
                    nc.vector.tensor_scalar(
                        x3q[:, cc, t * 1024 : (t + 1) * 1024], p,
                        b3[:, cc : cc + 1], 0.0,
                        op0=mybir.AluOpType.add, op1=mybir.AluOpType.max,
                    )

        # ---- q (full m), k (local n half), vT (local n half, transposed) ----
        q = big.tile([128, 2, N], bf16, tag="q")
        for cc in range(2):
            for t in range(4):
                p = ps.tile([128, 1024], f32, tag="ps", name="p_q")
                for ki in range(2):
                    for su in range(2):
                        nc.tensor.matmul(
                            p[:, su * 512 : (su + 1) * 512],
                            wqt[:, ki, cc * 128 : (cc + 1) * 128],
                            x3q[:, ki, t * 1024 + su * 512 : t * 1024 + (su + 1) * 512],
                            start=(ki == 0), stop=(ki == 1),
                        )
                nc.vector.tensor_scalar_add(
                    q[:, cc, t * 1024 : (t + 1) * 1024], p, bq[:, cc : cc + 1]
                )
        k_ = big.tile([128, 2, NH], bf16, tag="k")
        for cc in range(2):
            for t in range(2):
                p = ps.tile([128, 1024], f32, tag="ps", name="p_k")
                for ki in range(2):
                    for su in range(2):
                        nc.tensor.matmul(
                            p[:, su * 512 : (su + 1) * 512],
                            wkt[:, ki, cc * 128 : (cc + 1) * 128],
                            x3q[:, ki, t * 1024 + su * 512 : t * 1024 + (su + 1) * 512],
                            start=(ki == 0), stop=(ki == 1),
                        )
                nc.vector.tensor_scalar_add(
                    k_[:, cc, t * 1024 : (t + 1) * 1024], p, bk[:, cc : cc + 1]
                )
        # vT[n, c] = sum_ci x3[ci, n] WvT[ci, c] + bv[c]  (bias via K=1 matmul)
        vT = big.tile([128, NB, 256], bf16, tag="vT")
        for g in range(4):
            p = ps.tile([128, 1024], f32, tag="ps", name="p_vT")
            for j in range(4):
                nb = g * 4 + j
                nsl = slice(nb * 128, (nb + 1) * 128)
                o = slice(j * 256, (j + 1) * 256)
                nc.tensor.matmul(p[:, o], x3q[:, 0, nsl], wvt[:, 0, :], start=True, stop=False)
                nc.tensor.matmul(p[:, o], x3q[:, 1, nsl], wvt[:, 1, :], start=False, stop=False)
                nc.tensor.matmul(p[:, o], ones, bv, start=False, stop=True)
            nc.vector.tensor_copy(vT[:, g * 4 : (g + 1) * 4, :], p)

        # ---- conv input: x3c is x3q in the rolled frame — local window row
        #      j (0..35) = rolled row (j-2) mod 64; the per-core mtop/mbot
        #      masks zero the rows that are conv 'SAME' padding (the wrap rows
        #      land exactly where the masks already zero or keep correctly).
        x3c = big.tile([128, 2, 36, 66], bf16, tag="x3c")
        nc.vector.memset(x3c, 0.0)
        for cc in range(2):
            nc.vector.tensor_copy(
                x3c[:, cc, 2:36, 1:65],
                x3q[:, cc, 0 : 34 * 64].rearrange("p (a b) -> p a b", a=34),
            )
            nc.vector.tensor_copy(
                x3c[:, cc, 0:2, 1:65],
                x3q[:, cc, 62 * 64 : 64 * 64].rearrange("p (a b) -> p a b", a=2),
            )
        # zero the stem-of-zero border rows (true 'SAME' pad is zero in x3)
        for cc in range(2):
            nc.vector.tensor_scalar_mul(x3c[:, cc, 0:2, :], x3c[:, cc, 0:2, :], mtop)
            nc.vector.tensor_scalar_mul(x3c[:, cc, 34:36, :], x3c[:, cc, 34:36, :], mbot)

        wb1 = singles.tile([128, 2, 9, 256], bf16, tag="wb", name="wb1_sb")
        nc.scalar.dma_start(out=wb1, in_=wb1_d)
        y1p0 = big.tile([128, 34, 66], bf16, tag="h1")
        y1p1 = big.tile([128, 34, 66], bf16, tag="x_in")
        y1p_ = lambda ki: y1p0 if ki == 0 else y1p1
        nc.vector.memset(y1p0, 0.0)
        nc.vector.memset(y1p1, 0.0)

        # ---- S-loop / conv pieces (interleaved below) ----
        P0 = big.tile([128, NB // 2, N], bf16, tag="x3q")
        P1 = big.tile([128, NB // 2, N], bf16, tag="P1")

        def P_(nb):
            return (P0 if nb < NB // 2 else P1)[:, nb % (NB // 2), :]

        def s_block(nb):
            nsl = slice(nb * 128, (nb + 1) * 128)
            lp = singles.tile([128, 4], f32, tag="lp", bufs=4, name="lp")
            for t in range(4):
                p = ps.tile([128, 1024], f32, tag="ps", name="p_s")
                for ki in range(2):
                    for su in range(2):
                        o = t * 1024 + su * 512
                        nc.tensor.matmul(
                            p[:, su * 512 : (su + 1) * 512],
                            k_[:, ki, nsl], q[:, ki, o : o + 512],
                            start=(ki == 0), stop=(ki == 1),
                        )
                nc.scalar.activation(
                    P_(nb)[:, t * 1024 : (t + 1) * 1024], p, AF.Exp,
                    accum_out=lp[:, t : t + 1],
                )
            nc.vector.reduce_sum(out=lall[:, nb : nb + 1], in_=lp, axis=AX.X)
            nc.vector.reciprocal(rl[:, nb : nb + 1], lall[:, nb : nb + 1])
            nc.vector.tensor_scalar_mul(vT[:, nb, :], vT[:, nb, :], rl[:, nb : nb + 1])

        def conv1_group(cc, y1row0):
            """32 y1-rows as 2 psum tiles; each LDWEIGHTS feeds 4 matmuls."""
            pA = ps.tile([128, 1024], f32, tag="ps", name="p_c1a")
            pB = ps.tile([128, 1024], f32, tag="ps", name="p_c1b")
            for kt in range(18):
                ki, tap = kt // 9, kt % 9
                dh, dw = tap // 3, tap % 3
                for ti, p in ((0, pA), (1, pB)):
                    for su in range(2):
                        r = y1row0 + ti * 16 + su * 8
                        nc.tensor.matmul(
                            p[:, su * 512 : (su + 1) * 512],
                            wb1[:, ki, tap, cc * 128 : (cc + 1) * 128],
                            x3c[:, ki, r - 1 + dh : r - 1 + dh + 8, dw : dw + 64],
                            start=(kt == 0), stop=(kt == 17),
                        )
            for ti, p in ((0, pA), (1, pB)):
                r = y1row0 + ti * 16
                nc.scalar.activation(
                    y1p_(cc)[:, r - 1 : r - 1 + 16, 1:65], p,
                    AF.Relu, bias=bb1[:, cc : cc + 1],
                )

        def conv1_tail(cc):  # y1 rows 33..34 (N=128)
            p = ps.tile([128, 1024], f32, tag="ps", name="p_c1t")
            for kt in range(18):
                ki, tap = kt // 9, kt % 9
                dh, dw = tap // 3, tap % 3
                nc.tensor.matmul(
                    p[:, 0:128],
                    wb1[:, ki, tap, cc * 128 : (cc + 1) * 128],
                    x3c[:, ki, 32 + dh : 34 + dh, dw : dw + 64],
                    start=(kt == 0), stop=(kt == 17),
                )
            nc.scalar.activation(
                y1p_(cc)[:, 32:34, 1:65], p[:, 0:128],
                AF.Relu, bias=bb1[:, cc : cc + 1],
            )

        def conv2_group(cc, orow0, wb2):
            pA = ps.tile([128, 1024], f32, tag="ps", name="p_c2a")
            pB = ps.tile([128, 1024], f32, tag="ps", name="p_c2b")
            for kt in range(18):
                ki, tap = kt // 9, kt % 9
                dh, dw = tap // 3, tap % 3
                for ti, p in ((0, pA), (1, pB)):
                    for su in range(2):
                        r = orow0 + ti * 16 + su * 8
                        nc.tensor.matmul(
                            p[:, su * 512 : (su + 1) * 512],
                            wb2[:, ki, tap, cc * 128 : (cc + 1) * 128],
                            y1p_(ki)[:, r - 2 + dh : r - 2 + dh + 8, dw : dw + 64],
                            start=(kt == 0), stop=(kt == 17),
                        )
            for ti, p in ((0, pA), (1, pB)):
                r = orow0 + ti * 16
                st = big.tile([128, 1024], f32, tag=("h2" if ti else "x3c"), name="st_c")
                nc.vector.tensor_scalar_add(st, p, bb2[:, cc : cc + 1])
                nc.sync.dma_start(
                    out=oc_d[cc * 128 : (cc + 1) * 128, (r - 2) * 64 : (r - 2) * 64 + 1024],
                    in_=st,
                )

        # ---- interleave: S blocks are ScalarE(exp)-paced; conv groups keep
        #      TensorE busy meanwhile ----
        s_block(0)
        s_block(1)
        conv1_group(0, 1)
        s_block(2)
        s_block(3)
        conv1_group(1, 1)
        s_block(4)
        s_block(5)
        conv1_tail(0)
        conv1_tail(1)
        s_block(6)
        for cc in range(2):
            nc.vector.tensor_scalar_mul(y1p_(cc)[:, 0, :], y1p_(cc)[:, 0, :], mtop)
            nc.vector.tensor_scalar_mul(y1p_(cc)[:, 33, :], y1p_(cc)[:, 33, :], mbot)
        wb2 = singles.tile([128, 2, 9, 256], bf16, tag="wb", name="wb2_sb")
        nc.scalar.dma_start(out=wb2, in_=wb2_d)
        s_block(7)
        s_block(8)
        conv2_group(0, 2, wb2)
        s_block(9)
        s_block(10)
        conv2_group(1, 2, wb2)
        s_block(11)
        s_block(12)
        s_block(13)
        s_block(14)
        s_block(15)

        # ---- attn_out partial = (v/l) @ P; all 4 psum slots per cc,
        #      weight-stationary over nb (each LDWEIGHTS feeds 8 matmuls) ----
        for cc in range(2):
            pt = [ps.tile([128, 1024], f32, tag="ps", name=f"p_at{t}") for t in range(4)]
            for nb in range(NB):
                for t in range(4):
                    for su in range(2):
                        o = t * 1024 + su * 512
                        nc.tensor.matmul(
                            pt[t][:, su * 512 : (su + 1) * 512],
                            vT[:, nb, cc * 128 : (cc + 1) * 128],
                            P_(nb)[:, o : o + 512],
                            start=(nb == 0), stop=(nb == NB - 1),
                        )
            for t in range(4):
                st = big.tile([128, 1024], f32, tag=("h2" if t % 2 else "x3c"), name="st_a")
                nc.vector.tensor_copy(st, pt[t])
                nc.sync.dma_start(
                    out=oa_d[cc * 128 : (cc + 1) * 128, t * 1024 : (t + 1) * 1024],
                    in_=st,
                )

    nc.compile()
    return nc


def _get_nc():
    if "nc" not in _CACHE:
        _CACHE["nc"] = _build_nc()
    return _CACHE["nc"]


def _make_in_maps(x, w1, b1, w2, b2, w3, b3, wb1, bb1, wb2, bb2,
                  wq, bq, wk, bk, wv, bv):
    bfc = lambda a: np.ascontiguousarray(np.asarray(a, np.float32).astype(ml_dtypes.bfloat16))
    f32c = lambda a: np.ascontiguousarray(np.asarray(a, np.float32))

    def qkv_t(w):  # [O, CI] -> lhsT/rhs chunks [128, 2, 256]
        return bfc(np.asarray(w, np.float32).T.reshape(2, 128, 256).transpose(1, 0, 2))

    def conv_t(wb):  # [O, I, 3, 3] -> [128 kip, 2 ki, 9 tap, 256 o]
        a = np.asarray(wb, np.float32).transpose(1, 0, 2, 3)  # [I, O, 3, 3]
        a = a.reshape(2, 128, 256, 9)                          # [ki, kip, o, tap]
        return bfc(a.transpose(1, 0, 3, 2))                    # [kip, ki, tap, o]

    def bias2(b):  # [256] -> [128, 2] (col cc = chunk cc)
        return f32c(np.asarray(b, np.float32).reshape(2, 128).T)

    wsb = np.zeros((128, 2240), np.float32)
    wsb[0:3, 0:64] = np.asarray(w1).T
    wsb[0:64, 64:192] = np.asarray(w2).T
    wsb[:, 192:448] = np.asarray(w3).T
    wsb[:, 448:960] = qkv_t(wq).astype(np.float32).reshape(128, 512)
    wsb[:, 960:1472] = qkv_t(wk).astype(np.float32).reshape(128, 512)
    wsb[:, 1472:1984] = qkv_t(wv).astype(np.float32).reshape(128, 512)
    wsb[0, 1984:2240] = np.asarray(bv)
    fsb = np.zeros((128, 14), np.float32)
    fsb[0:64, 0] = np.asarray(b1)
    fsb[:, 1] = np.asarray(b2)
    fsb[:, 2:4] = bias2(b3)
    fsb[:, 4:6] = bias2(bq)
    fsb[:, 6:8] = bias2(bk)
    fsb[:, 8:10] = bias2(bb1)
    fsb[:, 10:12] = bias2(bb2)
    # fsb[:, 12:14] = per-core mtop/mbot, filled below
    common = {
        "wsb": bfc(wsb),
        "wb1": conv_t(wb1),
        "wb2": conv_t(wb2),
    }

    xf = np.asarray(x, np.float32).reshape(B, 3, N)
    in_maps = []
    for core in range(8):
        b, h = core // 2, core % 2
        xq = bfc(np.roll(xf[b], -NH * h, axis=1))
        # conv window: global rows [32h-2, 32h+34), zero outside the image
        fc = fsb.copy()
        fc[:, 12] = 0.0 if h == 0 else 1.0
        fc[:, 13] = 1.0 if h == 0 else 0.0
        in_maps.append(dict(
            common,
            xq=xq,
            fsb=f32c(fc),
        ))
    return in_maps


def _gather(results, alpha, beta):
    a, bt = float(alpha), float(beta)
    out = np.empty((B, C, H, W), np.float32)
    for b in range(B):
        r0, r1 = results[2 * b], results[2 * b + 1]
        attn = r0["out_attn"] + np.roll(r1["out_attn"], NH, axis=1)
        conv = np.concatenate(
            [r0["out_conv"].reshape(C, 32, W), r1["out_conv"].reshape(C, 32, W)],
            axis=1,
        )
        out[b] = a * conv + bt * attn.reshape(C, H, W)
    return out


def _run(inputs, trace=False, **kw):
    from concourse import bass_utils

    nc = _get_nc()
    in_maps = _make_in_maps(
        inputs["x"], inputs["w1"], inputs["b1"], inputs["w2"], inputs["b2"],
        inputs["w3"], inputs["b3"], inputs["wb1"], inputs["bb1"],
        inputs["wb2"], inputs["bb2"], inputs["wq"], inputs["bq"],
        inputs["wk"], inputs["bk"], inputs["wv"], inputs["bv"],
    )
    res = bass_utils.run_bass_kernel_spmd(
        nc, in_maps, core_ids=list(range(8)), trace=trace, **kw
    )
    return _gather(res.results, inputs["alpha"], inputs["beta"]), res


def kernel(**inputs):
    out, _ = _run(inputs, trace=False)
    return out
